# revision 16
# baseline (speedup 1.0000x reference)
"""GCN (2-layer, PyG-style GCNConv) on 8 Trainium2 NeuronCores.

Strategy (graph/data parallel, per sharding hint):
- Nodes dealt snake-wise by degree to 8 cores (6250 each); edges live on
  their target node's core. Both layers aggregate at width 128.
- Message gather: SWDGE dma_gather (Q7 descriptor generation is the
  bottleneck at ~8ns/idx, so everything else is arranged around it):
  bf16 rows (256B), per-(window, group) tiles of 128 edges. Groups =
  {A, B}: sources whose owner-core-local slot is < 3072 (A) vs the rest,
  gathered from contiguous reordered copies xpA/xpB (layer 1) and from
  two half-AllGather outputs y2A/y2B (layer 2). Both copies have <32768
  rows, so int16 indices need no phase split, and the A-half collective
  is triggered mid-layer-1 (after window 5) so both collectives hide
  under gather work.
- Self-loops never gathered: host ships a feature-major x*dinv^2 copy of
  the core's own nodes; the self contribution is one DVE add per window.
- Scatter-add: PE matmuls in bf16. Per tile a host-built S matrix
  [128 edges x 32 cols] carries dinv[u]*dinv[v]; M_tile^T @ S_tile
  accumulates feature-major into a 512-col PSUM window (K=1 zero matmul
  clears each window). S + gather indices are SBUF-resident (loaded once,
  reused by both layers -- same tiles).
- Tile packing: synchronized greedy sweep across cores per (window,
  group): anchor c0 = min over cores of first unassigned target col,
  every core fills <=128 edges with cols in [c0, c0+32). Always feasible,
  ~95% slot occupancy.
- Dense: h = relu(W1^T agg + b1) per window (transient), y2 = W2^T h,
  all bf16 with fp32 PSUM. y2 kept in SBUF feature-major for the layer-2
  self-add (scaled by resident dinv^2 broadcast).
- Layer hand-off: y2 PE-transposed per window to node-major bf16
  y2_locA/y2_locB in DRAM; AllGather per half. Layer-2 A-tiles depend on
  cc_A (done mid-layer-1), B-tiles on cc_B.
- Output: bf16 out buffer PE-transposed, staged fp32, written node-major;
  host inverts the node permutation.
"""

import sys

sys.path.insert(0, '/opt/trn_rl_repo')

import math

import ml_dtypes
import numpy as np

N_NODES = 50000
N_CORES = 8
D = 128
DH = 256
WINDOW = 512
T = 32            # S-matrix columns (tile write span)
TILE_E = 128      # edges per tile (partition dim)
GCH = 64          # tiles per gather chunk
DMA_SCRATCH = 16384
LO = 32768        # int16 index limit
SPLIT_A = 3072    # local cols per core in the A half (6 windows)
NGRP = 2          # gather groups per window (A, B)


# ----------------------------------------------------------------------------
# Host-side graph preprocessing
# ----------------------------------------------------------------------------

def _prep_graph(edge_index):
    """Partition + pack the graph (self-loops handled separately).

    Returns (static, percore): `static` fully determines the device
    program, `percore` holds per-core input arrays."""
    n = N_NODES
    npc = n // N_CORES
    row = np.asarray(edge_index[0], dtype=np.int64)
    col = np.asarray(edge_index[1], dtype=np.int64)
    deg = np.bincount(col, minlength=n).astype(np.float64) + 1.0  # + self loop
    dinv = np.where(deg > 0, 1.0 / np.sqrt(deg), 0.0)

    # snake-deal nodes by degree to balance per-core, per-window edge counts
    order = np.argsort(-deg, kind='stable')
    rank = np.arange(n)
    rounds, posn = rank // N_CORES, rank % N_CORES
    cores_for_rank = np.where(rounds % 2 == 0, posn, N_CORES - 1 - posn)
    perm = np.empty(n, dtype=np.int64)      # global slot -> node id
    off = 0
    for c in range(N_CORES):
        nodes_c = order[cores_for_rank == c]
        perm[off:off + len(nodes_c)] = nodes_c
        off += len(nodes_c)
    assert off == n
    inv = np.empty(n, dtype=np.int64)       # node id -> global slot
    inv[perm] = np.arange(n)

    src_slot = inv[row]
    dst_slot = inv[col]
    dst_core = dst_slot // npc
    dst_local = dst_slot % npc
    norm = (dinv[row] * dinv[col]).astype(np.float32)

    n_win = math.ceil(npc / WINDOW)
    wlens = [min(WINDOW, npc - w * WINDOW) for w in range(n_win)]

    # ---- per (core, window, group) edge lists sorted by local col ----
    # group A: source in first SPLIT_A local slots of its owner core
    # (gathered from the contiguous a-ordered copy xpA / y2A); group B:
    # the rest (xpB / y2B). Both copies have <32768 rows -> single-phase
    # int16 indices, and the layer-1 -> layer-2 hand-off collective can
    # be split per half and hidden under gather work.
    sa, sb = SPLIT_A, npc - SPLIT_A
    ewg = {}
    for c in range(N_CORES):
        mc = dst_core == c
        ec, es, en = dst_local[mc], src_slot[mc], norm[mc]
        sloc = es % npc
        gid = np.where(sloc < sa, 0, 1)
        gidx = np.where(sloc < sa, (es // npc) * sa + sloc,
                        (es // npc) * sb + (sloc - sa))
        wi = ec // WINDOW
        for w in range(n_win):
            for g in range(NGRP):
                m = (wi == w) & (gid == g)
                cols = ec[m] - w * WINDOW
                o = np.argsort(cols, kind='stable')
                ewg[(c, w, g)] = (cols[o], gidx[m][o], en[m][o])

    # ---- synchronized greedy sweep per (window, group) ----
    # anchors shared across cores; each core fills <=128 edges with
    # col in [c0, c0+T) per anchor. Always feasible.
    anchors_all = {}
    assign = {}    # (c, w, g) -> (tile_of, slot) per edge
    ntiles = {}
    for w in range(n_win):
        wlen = wlens[w]
        for g in range(NGRP):
            cols_by_core = [ewg[(c, w, g)][0] for c in range(N_CORES)]
            nedge = [len(x) for x in cols_by_core]
            ptr = [0] * N_CORES
            tile_of = [np.empty(ne, dtype=np.int64) for ne in nedge]
            slot = [np.empty(ne, dtype=np.int64) for ne in nedge]
            anchors = []
            while any(ptr[c] < nedge[c] for c in range(N_CORES)):
                c0 = min(cols_by_core[c][ptr[c]]
                         for c in range(N_CORES) if ptr[c] < nedge[c])
                c0 = min(c0, max(wlen - T, 0))
                j = len(anchors)
                anchors.append(int(c0))
                for c in range(N_CORES):
                    p0 = ptr[c]
                    if p0 >= nedge[c]:
                        continue
                    hi = np.searchsorted(cols_by_core[c], c0 + T, side='left')
                    take = min(TILE_E, hi - p0)
                    if take > 0:
                        tile_of[c][p0:p0 + take] = j
                        slot[c][p0:p0 + take] = np.arange(take)
                        ptr[c] = p0 + take
            anchors_all[(w, g)] = anchors
            ntiles[(w, g)] = len(anchors)
            for c in range(N_CORES):
                assign[(c, w, g)] = (tile_of[c], slot[c])

    # ---- global tile layout in execution order; chunks ----
    tile_base = {}
    cs_table = []
    chunks = {}        # (w, g) -> list of (global_tile0, k)
    tot = 0
    for w in range(n_win):
        for g in range(NGRP):
            nt = ntiles[(w, g)]
            tile_base[(w, g)] = tot
            cs_table.extend(anchors_all[(w, g)])
            ch = []
            j = 0
            while j < nt:
                k = min(GCH, nt - j)
                ch.append((tot + j, k))
                j += k
            chunks[(w, g)] = ch
            tot += nt

    smat = np.zeros((N_CORES, 128, tot, T), dtype=ml_dtypes.bfloat16)
    gidx_lin = np.zeros((N_CORES, tot, TILE_E), dtype=np.int16)
    for c in range(N_CORES):
        for w in range(n_win):
            for g in range(NGRP):
                cols, gi, en = ewg[(c, w, g)]
                if len(cols) == 0:
                    continue
                tof, slo = assign[(c, w, g)]
                base = tile_base[(w, g)]
                grid = np.asarray(anchors_all[(w, g)], dtype=np.int64)
                gt = base + tof
                coff = cols - grid[tof]
                assert (coff >= 0).all() and (coff < T).all()
                smat[c, slo, gt, coff] = en.astype(ml_dtypes.bfloat16)
                gidx_lin[c, gt, slo] = gi.astype(np.int16)

    # wrap indices: linear i -> partition i%16, col i//16; replicate x8
    gidx_w = np.zeros((N_CORES, 128, 8 * tot), dtype=np.int16)
    for c in range(N_CORES):
        lin = gidx_lin[c].reshape(tot * TILE_E)
        arr = lin.reshape(8 * tot, 16).T        # [16, 8*tot]
        gidx_w[c] = np.tile(arr, (8, 1))

    occ = (800000) / (tot * 8 * TILE_E / 2)  # rough per-layer occupancy
    static = dict(npc=npc, n_win=n_win, wlens=wlens, tot=tot,
                  cs_table=cs_table, chunks=chunks, ntiles=ntiles, occ=occ)
    percore = dict(smat=smat, gidx=gidx_w, perm=perm, dinv=dinv)
    return static, percore


# ----------------------------------------------------------------------------
# Device program
# ----------------------------------------------------------------------------

_CACHE = {}
_LAST = {}   # stash of the last BassKernelResults (test harness reads it)


def _build_program(st):
    import concourse.bacc as bacc
    import concourse.mybir as mybir
    import concourse.tile as tile
    from concourse.tile_rust import add_dep_helper

    npc, n_win, wlens, tot = st['npc'], st['n_win'], st['wlens'], st['tot']
    chunks = st['chunks']
    cs = st['cs_table']
    AF = mybir.ActivationFunctionType
    f32 = mybir.dt.float32
    bf = mybir.dt.bfloat16

    sa, sb = SPLIT_A, npc - SPLIT_A
    na, nb = N_CORES * sa, N_CORES * sb
    nwa = SPLIT_A // WINDOW            # windows in the A half
    nc = bacc.Bacc("TRN2", target_bir_lowering=False, num_devices=N_CORES,
                   dynamic_dma_scratch_size=DMA_SCRATCH)
    xpa = nc.dram_tensor("xpa", [na, D], bf, kind="ExternalInput")
    xpb = nc.dram_tensor("xpb", [nb, D], bf, kind="ExternalInput")
    xfms_dram = nc.dram_tensor("xfms", [128, n_win * WINDOW], bf,
                               kind="ExternalInput")
    dinv2_dram = nc.dram_tensor("dinv2", [128, n_win * WINDOW], bf,
                                kind="ExternalInput")
    s_dram = nc.dram_tensor("smat", [128, tot, T], bf, kind="ExternalInput")
    gi_dram = nc.dram_tensor("gidx", [128, 8 * tot], mybir.dt.int16,
                             kind="ExternalInput")
    w1_dram = nc.dram_tensor("w1", [D, DH], bf, kind="ExternalInput")
    b1_dram = nc.dram_tensor("b1", [128, 2], f32, kind="ExternalInput")
    w2_dram = nc.dram_tensor("w2", [128, 2, D], bf, kind="ExternalInput")
    b2_dram = nc.dram_tensor("b2", [128, 1], f32, kind="ExternalInput")
    id_dram = nc.dram_tensor("ident", [128, 128], bf, kind="ExternalInput")
    out_dram = nc.dram_tensor("out", [npc, D], f32, kind="ExternalOutput")
    y2_loca = nc.dram_tensor("y2_loca", [sa, D], bf)
    y2_locb = nc.dram_tensor("y2_locb", [sb, D], bf)
    y2a = nc.dram_tensor("y2a", [na, D], bf, addr_space="Shared")
    y2b = nc.dram_tensor("y2b", [nb, D], bf, addr_space="Shared")

    ncols = n_win * WINDOW

    with tile.TileContext(nc) as tc:
        with (
            tc.tile_pool(name="const", bufs=1) as constp,
            tc.tile_pool(name="big", bufs=1) as bigp,
            tc.tile_pool(name="mp", bufs=3) as mp,
            tc.tile_pool(name="hp", bufs=2) as hp,
            tc.tile_pool(name="tv", bufs=2) as tvp,
            tc.tile_pool(name="stage", bufs=2) as stagep,
            tc.tile_pool(name="stagef", bufs=2) as stagefp,
            tc.tile_pool(name="psA", bufs=2, space="PSUM") as psA,
            tc.tile_pool(name="psD", bufs=2, space="PSUM") as psD,
            tc.tile_pool(name="psT", bufs=2, space="PSUM") as psT,
        ):
            idx_sb = bigp.tile([128, 8 * tot], mybir.dt.int16)
            nc.sync.dma_start(idx_sb[:], gi_dram[:])
            w1_sb = constp.tile([128, DH], bf)
            nc.sync.dma_start(w1_sb[:], w1_dram[:])
            w2_sb = constp.tile([128, 2, D], bf)
            nc.sync.dma_start(w2_sb[:], w2_dram[:])
            b1_sb = constp.tile([128, 2], f32)
            nc.sync.dma_start(b1_sb[:], b1_dram[:])
            b2_sb = constp.tile([128, 1], f32)
            nc.sync.dma_start(b2_sb[:], b2_dram[:])
            id_sb = constp.tile([128, 128], bf)
            nc.sync.dma_start(id_sb[:], id_dram[:])
            xfms_sb = constp.tile([128, ncols], bf)
            nc.scalar.dma_start(xfms_sb[:], xfms_dram[:])
            dinv2_sb = constp.tile([128, ncols], bf)
            nc.scalar.dma_start(dinv2_sb[:], dinv2_dram[:])
            z1 = constp.tile([1, WINDOW], bf)
            nc.vector.memset(z1[:], 0.0)
            s_sb = bigp.tile([128, tot, T], bf)
            nc.scalar.dma_start(s_sb[:], s_dram[:])

            agg = bigp.tile([128, ncols], bf)
            y2 = bigp.tile([128, ncols], bf)
            outf = bigp.tile([128, ncols], bf)

            def aggregate(bases, drain_fn, gather_deps):
                """bases: per-group gather source APs; gather_deps: per-group
                list of instructions each gather must wait on."""
                for w in range(n_win):
                    wlen = wlens[w]
                    ps = psA.tile([128, WINDOW], f32)
                    nc.tensor.matmul(ps[:], z1[:1, 0:128], z1[:1, :],
                                     start=True, stop=False,
                                     skip_group_check=True)
                    last_g = None
                    for g in range(NGRP - 1, -1, -1):
                        if chunks[(w, g)]:
                            ch = chunks[(w, g)][-1]
                            last_g = ch[0] + ch[1] - 1
                            break
                    for g in range(NGRP):
                        base = bases[g]
                        for (g0, k) in chunks[(w, g)]:
                            m = mp.tile([128, GCH, D], bf)
                            gi = nc.gpsimd.dma_gather(
                                out_ap=m[:, :k, :],
                                in_ap=base,
                                idxs_ap=idx_sb[:, 8 * g0:8 * (g0 + k)],
                                num_idxs=TILE_E * k,
                                num_idxs_reg=TILE_E * k,
                                elem_size=D,
                                single_packet=False,
                            )
                            for dep in gather_deps[g]:
                                add_dep_helper(gi.ins, dep.ins,
                                               reason="gather dep")
                            for j in range(k):
                                gt = g0 + j
                                c0 = cs[gt]
                                nc.tensor.matmul(
                                    ps[:, c0:c0 + T],
                                    m[:, j, :],
                                    s_sb[:, gt, :],
                                    start=False, stop=(gt == last_g),
                                    skip_group_check=True,
                                )
                    drain_fn(w, wlen, ps)
                    del ps

            # ---------------- layer 1 ----------------
            def drain1(w, wlen, ps):
                c0 = w * WINDOW
                nc.vector.tensor_copy(agg[:, c0:c0 + wlen], ps[:, :wlen])
                nc.vector.tensor_tensor(
                    out=agg[:, c0:c0 + wlen],
                    in0=agg[:, c0:c0 + wlen],
                    in1=xfms_sb[:, c0:c0 + wlen],
                    op=mybir.AluOpType.add)
                # dense: h = relu(W1^T agg + b1); y2 = W2^T h (feature-major)
                h = hp.tile([128, 2, WINDOW], bf)
                for half in (0, 1):
                    psd = psD.tile([128, WINDOW], f32)
                    nc.tensor.matmul(psd[:, :wlen],
                                     w1_sb[:, half * 128:(half + 1) * 128],
                                     agg[:, c0:c0 + wlen],
                                     start=True, stop=True)
                    nc.scalar.activation(h[:, half, :wlen], psd[:, :wlen],
                                         AF.Relu,
                                         bias=b1_sb[:, half:half + 1])
                psd = psD.tile([128, WINDOW], f32)
                nc.tensor.matmul(psd[:, :wlen], w2_sb[:, 0, :],
                                 h[:, 0, :wlen], start=True, stop=False)
                nc.tensor.matmul(psd[:, :wlen], w2_sb[:, 1, :],
                                 h[:, 1, :wlen], start=False, stop=True)
                nc.vector.tensor_copy(y2[:, c0:c0 + wlen], psd[:, :wlen])

            # transpose one 512-col window of a feature-major SBUF buffer
            # to node-major DRAM rows [r0, r0+wlen) of dst_dram
            def writeback_win(src_sb, w, dst_dram, r0, stg_pool, stg_dt):
                dmas = []
                wlen = min(WINDOW, npc - w * WINDOW)
                nblk = math.ceil(wlen / 128)
                stg = stg_pool.tile([128, 4, 128], stg_dt)
                full = wlen == 512
                for bi in range(nblk):
                    c0 = w * WINDOW + bi * 128
                    blen = min(128, npc - c0)
                    pt = psT.tile([128, 128], bf)
                    nc.tensor.transpose(pt[:blen, :],
                                        src_sb[:, c0:c0 + blen],
                                        id_sb[:])
                    nc.vector.tensor_copy(stg[:blen, bi, :], pt[:blen, :])
                if full:
                    dv = dst_dram[r0:r0 + 512, :].rearrange(
                        "(j p) f -> p j f", p=128)
                    dmas.append(nc.sync.dma_start(dv, stg[:]))
                else:
                    for bi in range(nblk):
                        blen = min(128, npc - (w * WINDOW + bi * 128))
                        dmas.append(nc.sync.dma_start(
                            dst_dram[r0 + bi * 128:r0 + bi * 128 + blen, :],
                            stg[:blen, bi, :]))
                return dmas

            wb_a, wb_b = [], []
            ccs = {}

            def emit_cc(name, src_t, dst_t, wbs):
                cc = nc.gpsimd.collective_compute(
                    "AllGather",
                    mybir.AluOpType.bypass,
                    replica_groups=[list(range(N_CORES))],
                    ins=[src_t[:]],
                    outs=[dst_t[:]],
                )
                for d in wbs:
                    add_dep_helper(cc.ins, d.ins, reason="cc waits y2 wb")
                ccs[name] = cc

            def drain1_wb(w, wlen, ps):
                drain1(w, wlen, ps)
                if w < nwa:
                    wb_a.extend(writeback_win(
                        y2, w, y2_loca, w * WINDOW, stagep, bf))
                    if w == nwa - 1:
                        # trigger the A-half AllGather mid-stream so it
                        # overlaps the remaining layer-1 gathers
                        emit_cc('a', y2_loca, y2a, wb_a)
                else:
                    wb_b.extend(writeback_win(
                        y2, w, y2_locb, w * WINDOW - sa, stagep, bf))
                    if w == n_win - 1:
                        emit_cc('b', y2_locb, y2b, wb_b)

            aggregate([xpa[:], xpb[:]], drain1_wb, [[], []])
            cc_a, cc_b = ccs['a'], ccs['b']

            # ---------------- layer 2 ----------------
            def drain2(w, wlen, ps):
                c0 = w * WINDOW
                t0 = tvp.tile([128, WINDOW], bf)
                nc.vector.tensor_scalar_add(
                    out=t0[:, :wlen], in0=ps[:, :wlen],
                    scalar1=b2_sb[:, 0:1])
                t1 = tvp.tile([128, WINDOW], bf)
                nc.vector.tensor_tensor(
                    out=t1[:, :wlen], in0=y2[:, c0:c0 + wlen],
                    in1=dinv2_sb[:, c0:c0 + wlen], op=mybir.AluOpType.mult)
                nc.vector.tensor_tensor(
                    out=outf[:, c0:c0 + wlen], in0=t0[:, :wlen],
                    in1=t1[:, :wlen], op=mybir.AluOpType.add)

            def drain2_wb(w, wlen, ps):
                drain2(w, wlen, ps)
                writeback_win(outf, w, out_dram, w * WINDOW, stagefp, f32)

            aggregate([y2a[:], y2b[:]], drain2_wb, [[cc_a], [cc_b]])

    nc.compile()
    return nc


# ----------------------------------------------------------------------------
# Entry point
# ----------------------------------------------------------------------------

def kernel(x, edge_index, W1, b1, W2, b2):
    from concourse import bass_utils

    x = np.asarray(x, dtype=np.float32)
    edge_index = np.asarray(edge_index)
    W1 = np.asarray(W1, dtype=np.float32)
    b1 = np.asarray(b1, dtype=np.float32)
    W2 = np.asarray(W2, dtype=np.float32)
    b2 = np.asarray(b2, dtype=np.float32)

    key = edge_index.tobytes()[:64] + str(edge_index.shape).encode()
    cached = _CACHE.get('k')
    if cached is not None and cached[0] == key:
        st, pc, nc = cached[1], cached[2], cached[3]
    else:
        st, pc = _prep_graph(edge_index)
        nc = _build_program(st)
        _CACHE['k'] = (key, st, pc, nc)

    npc = st['npc']
    ncols = st['n_win'] * WINDOW
    perm = pc['perm']
    dinv = pc['dinv']
    bfd = ml_dtypes.bfloat16
    xp = np.ascontiguousarray(x[perm]).astype(bfd)
    # a-order: first SPLIT_A local slots of every core, concatenated
    xp3 = xp.reshape(N_CORES, npc, D)
    xpa = np.ascontiguousarray(xp3[:, :SPLIT_A].reshape(-1, D))
    xpb = np.ascontiguousarray(xp3[:, SPLIT_A:].reshape(-1, D))
    b1_in = np.ascontiguousarray(b1.reshape(2, 128).T).astype(np.float32)
    b2_in = np.ascontiguousarray(b2.reshape(128, 1)).astype(np.float32)
    w1_in = W1.astype(bfd)
    w2_in = np.ascontiguousarray(
        W2.reshape(2, 128, D).transpose(1, 0, 2)).astype(bfd)
    ident = np.eye(128, dtype=np.float32).astype(bfd)

    in_maps = []
    for c in range(N_CORES):
        perm_c = perm[c * npc:(c + 1) * npc]
        d2 = (dinv[perm_c] ** 2).astype(np.float32)
        xfms = np.zeros((128, ncols), dtype=bfd)
        xfms[:, :npc] = (x[perm_c] * d2[:, None]).T.astype(bfd)
        dinv2 = np.zeros((128, ncols), dtype=bfd)
        dinv2[:, :npc] = np.broadcast_to(
            d2[None, :], (128, npc)).astype(bfd)
        in_maps.append({
            "xpa": xpa,
            "xpb": xpb,
            "xfms": xfms,
            "dinv2": dinv2,
            "smat": pc['smat'][c],
            "gidx": pc['gidx'][c],
            "w1": w1_in,
            "b1": b1_in,
            "w2": w2_in,
            "b2": b2_in,
            "ident": ident,
        })

    res = bass_utils.run_bass_kernel_spmd(
        nc, in_maps, core_ids=list(range(N_CORES)))
    _LAST['res'] = res

    full = np.concatenate([res.results[c]["out"] for c in range(N_CORES)], 0)
    out = np.empty((N_NODES, D), dtype=np.float32)
    out[perm] = full
    return out
